# revision 2
# baseline (speedup 1.0000x reference)
"""BertSelfAttention on 8 Trainium2 NeuronCores (Bass/Tile).

Problem: B=4, S=2048, HID=768, NH=12, HD=64 (fp32).
    q/k/v = hs @ W{q,k,v}.T + b;  scores = q k^T / 8 + mask;  ctx = softmax(scores) v

Sharding: 8 cores = 4 batches x 2 head-groups of 6 heads. Core c handles
batch c//2, heads (c%2)*6..+6. No collectives; each core produces the
[2048, 384] slice out[b, :, hg*384:(hg+1)*384].

Per-core pipeline (all matmul contractions need the contracted dim on SBUF
partitions, so the host passes hs^T and W^T slices; bf16 inputs, fp32 PSUM):
  1. qT/kT [384(d), 2048] = wT-as-weights x hsT-streaming   (d packed 2 heads/tile)
  2. v  [2048(s), 6, 65]  = hsT-as-weights x wvT-streaming; col 64 = ones
     (ones column makes the probs@v matmul also emit softmax denominators)
  3. per head: scoresT[ki, qi] = kT-weights x qT  -> PSUM
     ACT: probs = exp(scoresT/8 + mask[ki]) -> bf16 SBUF   (per-partition bias)
     ctx[qi, 64+1] += probsT-as-bf16-weights x v'           (fp32 accum, 16 kt)
     DVE: ctx[:, :64] * recip(ctx[:, 64]) -> out tile
Softmax skips the max-subtraction (scores ~ N(0,1); exp is safe in fp32 and
softmax is shift-invariant, so this matches the reference).
"""

import numpy as np
import ml_dtypes

from concourse import bacc, tile
import concourse.mybir as mybir
from concourse.bass_utils import run_bass_kernel_spmd

B, S, HID, NH, HD = 4, 2048, 768, 12, 64
N_CORES = 8
NHC = NH // 2          # heads per core = 6
DG = NHC * HD          # per-core output width = 384
KC = HID // 128        # contraction chunks = 6
MT = DG // 128         # q/k M-tiles (2 heads each) = 3
NT = S // 128          # sequence tiles = 16
F32 = mybir.dt.float32
BF16 = mybir.dt.bfloat16
BF16NP = ml_dtypes.bfloat16


def build_tile(tc):
    nc = tc.nc
    hsT = nc.dram_tensor("hsT", (HID, S), BF16, kind="ExternalInput").ap()
    wqT = nc.dram_tensor("wqT", (HID, DG), BF16, kind="ExternalInput").ap()
    wkT = nc.dram_tensor("wkT", (HID, DG), BF16, kind="ExternalInput").ap()
    wvT = nc.dram_tensor("wvT", (HID, DG), BF16, kind="ExternalInput").ap()
    bq = nc.dram_tensor("bq", (128, MT), F32, kind="ExternalInput").ap()
    bk = nc.dram_tensor("bk", (128, MT), F32, kind="ExternalInput").ap()
    bvr = nc.dram_tensor("bvrow", (1, DG), BF16, kind="ExternalInput").ap()
    msk = nc.dram_tensor("mask", (128, NT), F32, kind="ExternalInput").ap()
    out = nc.dram_tensor("out", (S, DG), F32, kind="ExternalOutput").ap()

    from contextlib import ExitStack

    with ExitStack() as ctx:
        main = ctx.enter_context(tc.tile_pool(name="main", bufs=1))
        small = ctx.enter_context(tc.tile_pool(name="small", bufs=4))
        ps_s = ctx.enter_context(tc.tile_pool(name="ps_s", bufs=3, space="PSUM"))
        ps_c = ctx.enter_context(tc.tile_pool(name="ps_c", bufs=2, space="PSUM"))

        qT_sb = main.tile([128, MT, S], BF16)
        kT_sb = main.tile([128, MT, S], BF16)
        v_sb = main.tile([128, NT, NHC, HD + 1], BF16)
        ctx_sb = main.tile([128, NT, DG], F32)
        mask_sb = main.tile([128, NT], F32)
        bq_sb = main.tile([128, MT], F32)
        bk_sb = main.tile([128, MT], F32)

        nc.sync.dma_start(mask_sb[:], msk[:])
        nc.sync.dma_start(bq_sb[:], bq[:])
        nc.sync.dma_start(bk_sb[:], bk[:])
        nc.gpsimd.memset(v_sb[:, :, :, HD : HD + 1], 1.0)

        with tc.tile_pool(name="wpool", bufs=1) as wpool:
            hsT_sb = wpool.tile([128, KC, S], BF16)
            nc.sync.dma_start(hsT_sb[:], hsT.rearrange("(kc p) s -> p kc s", p=128))
            w_sbs = []
            for name, w in (("wq", wqT), ("wk", wkT), ("wv", wvT)):
                wsb = wpool.tile([128, KC, DG], BF16, tag=name)
                nc.sync.dma_start(wsb[:], w.rearrange("(kc p) d -> p kc d", p=128))
                w_sbs.append(wsb)
            wq_sb, wk_sb, wv_sb = w_sbs
            ones_sb = wpool.tile([1, 128], BF16)
            nc.vector.memset(ones_sb[:], 1.0)
            bvr_sb = wpool.tile([1, DG], BF16)
            nc.sync.dma_start(bvr_sb[:], bvr[:])

            # q/k projections: qT[d, s] = sum_c wqT[c, d] hsT[c, s]
            for wsb, dest, bias_sb in ((wq_sb, qT_sb, bq_sb), (wk_sb, kT_sb, bk_sb)):
                for mt in range(MT):
                    pst = [ps_s.tile([128, 1024], F32, tag="ps_s", name="pst")
                           for _ in range(2)]
                    for kc in range(KC):
                        for nch in range(4):
                            nc.tensor.matmul(
                                pst[nch // 2][:, (nch % 2) * 512 : (nch % 2) * 512 + 512],
                                wsb[:, kc, mt * 128 : (mt + 1) * 128],
                                hsT_sb[:, kc, nch * 512 : (nch + 1) * 512],
                                start=(kc == 0),
                                stop=(kc == KC - 1),
                            )
                    for nch in range(4):
                        nc.vector.tensor_scalar_add(
                            dest[:, mt, nch * 512 : (nch + 1) * 512],
                            pst[nch // 2][:, (nch % 2) * 512 : (nch % 2) * 512 + 512],
                            bias_sb[:, mt : mt + 1],
                        )

            # v projection: v[s, d] = sum_c hsT[c, s] wvT[c, d] + bv[d]
            for st in range(NT):
                pv = ps_c.tile([128, NHC, HD], F32, tag="ps_c")
                for kc in range(KC):
                    nc.tensor.matmul(
                        pv[:],
                        hsT_sb[:, kc, st * 128 : (st + 1) * 128],
                        wv_sb[:, kc, :],
                        start=(kc == 0),
                        stop=False,
                    )
                nc.tensor.matmul(pv[:], ones_sb[:], bvr_sb[:], start=False, stop=True)
                nc.vector.tensor_copy(v_sb[:, st, :, 0:HD], pv[:])

        # attention, one head at a time
        with tc.tile_pool(name="probs", bufs=2) as ppool:
            for h in range(NHC):
                mt, pb = h // 2, (h % 2) * 64
                probs = ppool.tile([128, NT, S], BF16, tag="probs")
                for kt in range(NT):
                    for half in range(2):
                        pst = ps_s.tile([128, 1024], F32, tag="ps_s")
                        for nch in range(2):
                            col = half * 1024 + nch * 512
                            nc.tensor.matmul(
                                pst[:, nch * 512 : (nch + 1) * 512],
                                kT_sb[pb : pb + 64, mt, kt * 128 : (kt + 1) * 128],
                                qT_sb[pb : pb + 64, mt, col : col + 512],
                            )
                        nc.scalar.activation(
                            probs[:, kt, half * 1024 : half * 1024 + 1024],
                            pst[:],
                            mybir.ActivationFunctionType.Exp,
                            bias=mask_sb[:, kt : kt + 1],
                            scale=0.125,
                        )
                for qt in range(NT):
                    pc = ps_c.tile([128, HD + 1], F32, tag="ps_c")
                    for kt in range(NT):
                        nc.tensor.matmul(
                            pc[:],
                            probs[:, kt, qt * 128 : (qt + 1) * 128],
                            v_sb[:, kt, h, :],
                            start=(kt == 0),
                            stop=(kt == NT - 1),
                        )
                    rcp = small.tile([128, 1], F32, tag="rcp")
                    nc.vector.reciprocal(rcp[:], pc[:, HD : HD + 1])
                    nc.vector.tensor_scalar_mul(
                        ctx_sb[:, qt, h * HD : (h + 1) * HD], pc[:, 0:HD], rcp[:]
                    )

        nc.sync.dma_start(out.rearrange("(t p) c -> p t c", p=128), ctx_sb[:])


_NC_CACHE = None


def get_nc():
    global _NC_CACHE
    if _NC_CACHE is None:
        nc = bacc.Bacc("TRN2", target_bir_lowering=False, debug=False,
                       num_devices=N_CORES)
        with tile.TileContext(nc) as tc:
            build_tile(tc)
        nc.compile()
        _NC_CACHE = nc
    return _NC_CACHE


def make_in_maps(hs, mask, Wq, bq, Wk, bk, Wv, bv):
    in_maps = []
    for c in range(N_CORES):
        b, hg = c // 2, c % 2
        hsl = slice(hg * DG, (hg + 1) * DG)
        in_maps.append({
            "hsT": np.ascontiguousarray(hs[b].T).astype(BF16NP),
            "wqT": np.ascontiguousarray(Wq[hsl].T).astype(BF16NP),
            "wkT": np.ascontiguousarray(Wk[hsl].T).astype(BF16NP),
            "wvT": np.ascontiguousarray(Wv[hsl].T).astype(BF16NP),
            "bq": np.ascontiguousarray(bq[hsl].reshape(MT, 128).T),
            "bk": np.ascontiguousarray(bk[hsl].reshape(MT, 128).T),
            "bvrow": bv[hsl].reshape(1, DG).astype(BF16NP),
            "mask": np.ascontiguousarray(mask[b, 0, 0].reshape(NT, 128).T),
        })
    return in_maps


def kernel(hidden_states, attention_mask, Wq, bq, Wk, bk, Wv, bv, **run_kwargs):
    hs = np.asarray(hidden_states, np.float32)
    mask = np.asarray(attention_mask, np.float32)
    Wq, bq = np.asarray(Wq, np.float32), np.asarray(bq, np.float32)
    Wk, bk = np.asarray(Wk, np.float32), np.asarray(bk, np.float32)
    Wv, bv = np.asarray(Wv, np.float32), np.asarray(bv, np.float32)

    nc = get_nc()
    in_maps = make_in_maps(hs, mask, Wq, bq, Wk, bk, Wv, bv)
    res = run_bass_kernel_spmd(nc, in_maps, list(range(N_CORES)), **run_kwargs)

    out = np.empty((B, S, HID), np.float32)
    for c in range(N_CORES):
        b, hg = c // 2, c % 2
        out[b, :, hg * DG : (hg + 1) * DG] = res.results[c]["out"]
    if run_kwargs:
        kernel.last_result = res
    return out


# revision 6
# speedup vs baseline: 1.0619x; 1.0619x over previous
"""BertSelfAttention on 8 Trainium2 NeuronCores (Bass/Tile).

Problem: B=4, S=2048, HID=768, NH=12, HD=64 (fp32).
    q/k/v = hs @ W{q,k,v}.T + b;  scores = q k^T / 8 + mask;  ctx = softmax(scores) v

Sharding: 8 cores = 4 batches x 2 head-groups of 6 heads. Core c handles
batch c//2, heads (c%2)*6..+6. No collectives; each core produces the
[2048, 384] slice out[b, :, hg*384:(hg+1)*384].

Per-core pipeline (matmul contractions need the contracted dim on SBUF
partitions, so the host passes hs^T and W^T slices; bf16 operands, fp32 PSUM):
  1. qT/kT [384(d), 2048] = wT-as-weights x hsT-streaming  (d packed 2 heads/tile)
  2. v  [2048(s), 6, 65]  = hsT-as-weights x wvT-streaming; col 64 = ones
     (the ones column makes the probs@v matmul also emit softmax denominators)
  3. per head: scoresT[ki, qi] = kT-weights x qT -> PSUM
     ACT: probs = exp(scoresT/8 + mask[ki]) -> bf16 SBUF  (per-partition bias)
     ctx[qi, 64+1] += probsT-as-bf16-weights x v'          (fp32 accum, 16 kt)
     DVE: ctx[:, :64] * recip(ctx[:, 64]) -> out tile

Emission is software-pipelined: the per-head scores+exp stream (ACT-paced) is
interleaved with "fill" units (remaining QKV projections during head 0, then
ctx of the previous head) so the in-order PE stream always has queued matmuls
while ACT works through the exps. SBUF cannot hold the QKV inputs plus two
probs buffers at once, so the second probs buffer's pool opens only after the
QKV input pool closes.

Softmax skips the max-subtraction (scores ~ N(0,1); exp is safe in fp32 and
softmax is shift-invariant, so this matches the reference).
"""

from collections import deque
from contextlib import ExitStack

import numpy as np
import ml_dtypes

from concourse import bacc, tile
import concourse.mybir as mybir
from concourse.bass_utils import run_bass_kernel_spmd

B, S, HID, NH, HD = 4, 2048, 768, 12, 64
N_CORES = 8
NHC = NH // 2          # heads per core = 6
DG = NHC * HD          # per-core output width = 384
KC = HID // 128        # contraction chunks = 6
MT = DG // 128         # q/k M-tiles (2 heads each) = 3
NT = S // 128          # sequence tiles = 16
F32 = mybir.dt.float32
BF16 = mybir.dt.bfloat16
BF16NP = ml_dtypes.bfloat16

SCORES_N = 512         # rhs columns per scores matmul (512 or 1024)


def build_tile(tc):
    nc = tc.nc
    hsT = nc.dram_tensor("hsT", (HID, S), BF16, kind="ExternalInput").ap()
    wqT = nc.dram_tensor("wqT", (HID, DG), BF16, kind="ExternalInput").ap()
    wkT = nc.dram_tensor("wkT", (HID, DG), BF16, kind="ExternalInput").ap()
    wvT = nc.dram_tensor("wvT", (HID, DG), BF16, kind="ExternalInput").ap()
    bq = nc.dram_tensor("bq", (128, MT), F32, kind="ExternalInput").ap()
    bk = nc.dram_tensor("bk", (128, MT), F32, kind="ExternalInput").ap()
    bvr = nc.dram_tensor("bvrow", (1, DG), BF16, kind="ExternalInput").ap()
    msk = nc.dram_tensor("mask", (128, NT), F32, kind="ExternalInput").ap()
    out = nc.dram_tensor("out", (S, DG), F32, kind="ExternalOutput").ap()
    out_r = out.rearrange("(t p) c -> p t c", p=128)

    with ExitStack() as stack:
        main = stack.enter_context(tc.tile_pool(name="main", bufs=1))
        small = stack.enter_context(tc.tile_pool(name="small", bufs=4))
        ppool_a = stack.enter_context(tc.tile_pool(name="probs_a", bufs=1))
        ps_s = stack.enter_context(tc.tile_pool(name="ps_s", bufs=3, space="PSUM"))
        ps_c = stack.enter_context(tc.tile_pool(name="ps_c", bufs=2, space="PSUM"))

        qT_sb = main.tile([128, MT, S], BF16)
        kT_sb = main.tile([128, MT, S], BF16)
        v_sb = main.tile([128, NT, NHC, HD + 1], BF16)
        ctx_sb = main.tile([128, NT, DG], F32)
        mask_sb = main.tile([128, NT], F32)
        bq_sb = main.tile([128, MT], F32)
        bk_sb = main.tile([128, MT], F32)

        nc.sync.dma_start(mask_sb[:], msk[:])
        nc.sync.dma_start(bq_sb[:], bq[:])
        nc.sync.dma_start(bk_sb[:], bk[:])
        nc.gpsimd.memset(v_sb[:, :, :, HD : HD + 1], 1.0)

        fill = deque()

        def ctx_unit(h, probs, qt, last_head):
            """ctx[qi-tile, h] = sum_kt probsT-weights x v'; normalize; (+ DMA)."""
            def emit():
                pc = ps_c.tile([128, HD + 1], F32, tag="ps_c", name="pc")
                for kt in range(NT):
                    nc.tensor.matmul(
                        pc[:],
                        probs[:, kt, qt * 128 : (qt + 1) * 128],
                        v_sb[:, kt, h, :],
                        start=(kt == 0),
                        stop=(kt == NT - 1),
                    )
                rcp = small.tile([128, 1], F32, tag="rcp", name="rcp")
                nc.vector.reciprocal(rcp[:], pc[:, HD : HD + 1])
                nc.vector.tensor_scalar_mul(
                    ctx_sb[:, qt, h * HD : (h + 1) * HD], pc[:, 0:HD], rcp[:]
                )
                if last_head:
                    nc.sync.dma_start(out_r[:, qt, :], ctx_sb[:, qt, :])
            return emit

        def scores_head(h, probs):
            mt, pb = h // 2, (h % 2) * 64
            for kt in range(NT):
                for half in range(2):
                    pst = ps_s.tile([128, 1024], F32, tag="ps_s", name="pst")
                    for nch in range(1024 // SCORES_N):
                        col = half * 1024 + nch * SCORES_N
                        nc.tensor.matmul(
                            pst[:, nch * SCORES_N : (nch + 1) * SCORES_N],
                            kT_sb[pb : pb + 64, mt, kt * 128 : (kt + 1) * 128],
                            qT_sb[pb : pb + 64, mt, col : col + SCORES_N],
                        )
                    nc.scalar.activation(
                        probs[:, kt, half * 1024 : half * 1024 + 1024],
                        pst[:],
                        mybir.ActivationFunctionType.Exp,
                        bias=mask_sb[:, kt : kt + 1],
                        scale=0.125,
                    )
                # drain fill work under the ACT-paced exp stream
                budget = 2 if h == 0 else 1
                for _ in range(budget):
                    if fill:
                        fill.popleft()()
            for qt in range(NT):
                fill.append(ctx_unit(h, probs, qt, h == NHC - 1))

        with tc.tile_pool(name="wpool", bufs=1) as wpool:
            hsT_sb = wpool.tile([128, KC, S], BF16)
            nc.sync.dma_start(hsT_sb[:], hsT.rearrange("(kc p) s -> p kc s", p=128))
            w_sbs = []
            for name, w in (("wq", wqT), ("wk", wkT), ("wv", wvT)):
                wsb = wpool.tile([128, KC, DG], BF16, tag=name, name=name)
                nc.sync.dma_start(wsb[:], w.rearrange("(kc p) d -> p kc d", p=128))
                w_sbs.append(wsb)
            wq_sb, wk_sb, wv_sb = w_sbs
            ones_sb = wpool.tile([1, 128], BF16)
            nc.vector.memset(ones_sb[:], 1.0)
            bvr_sb = wpool.tile([1, DG], BF16)
            nc.sync.dma_start(bvr_sb[:], bvr[:])

            def qk_unit(wsb, dest, bias_sb, mt):
                """qT[d, s] M-tile = sum_c wT[c, d-tile] hsT[c, s]  (+ bias)."""
                def emit():
                    pst = [ps_s.tile([128, 1024], F32, tag="ps_s", name="pst")
                           for _ in range(2)]
                    for kc in range(KC):
                        for nch in range(4):
                            nc.tensor.matmul(
                                pst[nch // 2][
                                    :, (nch % 2) * 512 : (nch % 2) * 512 + 512
                                ],
                                wsb[:, kc, mt * 128 : (mt + 1) * 128],
                                hsT_sb[:, kc, nch * 512 : (nch + 1) * 512],
                                start=(kc == 0),
                                stop=(kc == KC - 1),
                            )
                    for nch in range(4):
                        nc.vector.tensor_scalar_add(
                            dest[:, mt, nch * 512 : (nch + 1) * 512],
                            pst[nch // 2][:, (nch % 2) * 512 : (nch % 2) * 512 + 512],
                            bias_sb[:, mt : mt + 1],
                        )
                return emit

            def v_unit(st):
                """v[s-tile, 6, 64] = sum_c hsT[c, s-tile] wvT[c, :] + bv."""
                def emit():
                    pv = ps_c.tile([128, NHC, HD], F32, tag="ps_c", name="pv")
                    for kc in range(KC):
                        nc.tensor.matmul(
                            pv[:],
                            hsT_sb[:, kc, st * 128 : (st + 1) * 128],
                            wv_sb[:, kc, :],
                            start=(kc == 0),
                            stop=False,
                        )
                    nc.tensor.matmul(
                        pv[:], ones_sb[:], bvr_sb[:], start=False, stop=True
                    )
                    nc.vector.tensor_copy(v_sb[:, st, :, 0:HD], pv[:])
                return emit

            # q/k M-tile 0 emitted inline so head 0 can start; the rest
            # becomes fill work drained underneath head 0's exp stream.
            qk_unit(wq_sb, qT_sb, bq_sb, 0)()
            qk_unit(wk_sb, kT_sb, bk_sb, 0)()
            for mt in (1, 2):
                fill.append(qk_unit(wq_sb, qT_sb, bq_sb, mt))
                fill.append(qk_unit(wk_sb, kT_sb, bk_sb, mt))
            for st in range(NT):
                fill.append(v_unit(st))

            probs_a = ppool_a.tile([128, NT, S], BF16, tag="probs", name="probs_a")
            scores_head(0, probs_a)
            # force-drain leftover QKV units (they need wpool tiles); the
            # trailing NT entries are head 0's ctx units, which don't.
            while len(fill) > NT:
                fill.popleft()()

        # wpool closed: hsT/W space free -> second probs buffer fits.
        with tc.tile_pool(name="probs_b", bufs=1) as ppool_b:
            for h in range(1, NHC):
                pool = ppool_b if h % 2 else ppool_a
                probs = pool.tile([128, NT, S], BF16, tag="probs",
                                  name=f"probs_{h}")
                scores_head(h, probs)
            while fill:
                fill.popleft()()


_NC_CACHE = None


def get_nc():
    global _NC_CACHE
    if _NC_CACHE is None:
        nc = bacc.Bacc("TRN2", target_bir_lowering=False, debug=False,
                       num_devices=N_CORES)
        with tile.TileContext(nc) as tc:
            build_tile(tc)
        nc.compile()
        _NC_CACHE = nc
    return _NC_CACHE


def make_in_maps(hs, mask, Wq, bq, Wk, bk, Wv, bv):
    in_maps = []
    for c in range(N_CORES):
        b, hg = c // 2, c % 2
        hsl = slice(hg * DG, (hg + 1) * DG)
        in_maps.append({
            "hsT": np.ascontiguousarray(hs[b].T).astype(BF16NP),
            "wqT": np.ascontiguousarray(Wq[hsl].T).astype(BF16NP),
            "wkT": np.ascontiguousarray(Wk[hsl].T).astype(BF16NP),
            "wvT": np.ascontiguousarray(Wv[hsl].T).astype(BF16NP),
            "bq": np.ascontiguousarray(bq[hsl].reshape(MT, 128).T),
            "bk": np.ascontiguousarray(bk[hsl].reshape(MT, 128).T),
            "bvrow": bv[hsl].reshape(1, DG).astype(BF16NP),
            "mask": np.ascontiguousarray(mask[b, 0, 0].reshape(NT, 128).T),
        })
    return in_maps


def kernel(hidden_states, attention_mask, Wq, bq, Wk, bk, Wv, bv, **run_kwargs):
    hs = np.asarray(hidden_states, np.float32)
    mask = np.asarray(attention_mask, np.float32)
    Wq, bq = np.asarray(Wq, np.float32), np.asarray(bq, np.float32)
    Wk, bk = np.asarray(Wk, np.float32), np.asarray(bk, np.float32)
    Wv, bv = np.asarray(Wv, np.float32), np.asarray(bv, np.float32)

    nc = get_nc()
    in_maps = make_in_maps(hs, mask, Wq, bq, Wk, bk, Wv, bv)
    res = run_bass_kernel_spmd(nc, in_maps, list(range(N_CORES)), **run_kwargs)

    out = np.empty((B, S, HID), np.float32)
    for c in range(N_CORES):
        b, hg = c // 2, c % 2
        out[b, :, hg * DG : (hg + 1) * DG] = res.results[c]["out"]
    if run_kwargs:
        kernel.last_result = res
    return out
